# revision 20
# baseline (speedup 1.0000x reference)
"""Trainium2 Bass kernel for a DANet-style DualAttentionBlock.

Full-input contract: kernel(**inputs) takes the complete tensors and returns
the complete [4, 256, 64, 64] output.

Sharding: 8 NeuronCores = 4 samples x 2 row-halves (32 rows each). Each core:
  - computes q for its 34-row query window (32 real + 1 halo row each side;
    out-of-range halo rows are zero-padded on host and masked out on device),
  - computes k/v for all 4096 key positions of its sample,
  - position attention in transposed orientation (keys on partitions) so the
    softmax denominator folds into matmuls and no attention transpose is
    needed,
  - full channel attention (energy is symmetric; softmax row-wise, then a
    256x256 PE transpose),
  - the 3x3 fusion conv for its 32 output rows (halo rows give exact edges),
  - partial BN stats; one fused 8-core AllReduce yields exact train-mode
    batch stats; normalize + ReLU on device.

Precision strategy: x ships as bf16 (k/v gen, channel attention all bf16 —
the channel softmax is diagonal-dominated so bf16 energies are safe); q/k and
the spatial energies in fp16 with a 4x-replicated K=128 contraction (q
pre-scaled by 1/4); attention weights and the fusion conv in bf16. The PV
matmul runs in [query, channel] orientation against V augmented with a ones
column, so the softmax denominator falls out of column 256 for free; results
are PE-transposed back into the conv layout. All small constants arrive in
three packed tensors (one DMA each) so descriptor generation doesn't
serialize the head; the BN statistics use a single [128,4] AllReduce.
"""

import os
import sys
import types

for _p in ("/opt/trn_rl_repo",):
    if _p not in sys.path:
        sys.path.append(_p)

import numpy as np
import ml_dtypes  # noqa: F401

import concourse.bass as bass
import concourse.mybir as mybir
import concourse.tile as tile
from concourse import bacc
from concourse.bass_utils import run_bass_kernel_spmd
from concourse.masks import make_identity

F32 = mybir.dt.float32
F32R = mybir.dt.float32r
BF16 = mybir.dt.bfloat16
F16 = mybir.dt.float16
AF = mybir.ActivationFunctionType
ALU = mybir.AluOpType
AX = mybir.AxisListType

B, C, H, W = 4, 256, 64, 64
N = H * W              # 4096 key positions
WROWS = 34             # query-window rows (1 halo + 32 real + 1 halo)
WQ = WROWS * W         # 2176 window positions
BN_EPS = 1e-5
NPOS = float(B * H * W)  # BN normalizer (16384)

# i-chunks for the attention phase; all widths >= 256 keep fp16 at full rate
CHUNKS = [(0, 512), (512, 512), (1024, 512), (1536, 384), (1920, 256)]

# spack (f32 consts pack) column layout
SP_BQ = 0            # bq replicated /4        [.., 1]
SP_BK = 1            # bk replicated           [.., 1]
SP_MASK = 2          # window mask, 17 blocks  [.., 17]
SP_BNG = 19          # bn gamma (2 oh halves)  [.., 2]
SP_BNB = 21          # bn beta  (2 oh halves)  [.., 2]
SP_BV = 23           # v bias row + denom cols, broadcast to 128 [.., 258]
SP_COLS = SP_BV + C + 2

LAST_RESULT = {}


def _build(gpa: float, gca: float):
    nc = bacc.Bacc("TRN2", target_bir_lowering=False)

    xq_d = nc.dram_tensor("xq", [C, WQ], F16, kind="ExternalInput")
    xb_d = nc.dram_tensor("xb", [C, N], BF16, kind="ExternalInput")
    wf16_d = nc.dram_tensor("wf16", [128, 256], F16, kind="ExternalInput")
    wbf_d = nc.dram_tensor("wbf", [128, 772], BF16, kind="ExternalInput")
    spack_d = nc.dram_tensor("spack", [128, SP_COLS], F32, kind="ExternalInput")
    wft_d = nc.dram_tensor("wft", [4, 128, 2304], BF16, kind="ExternalInput")
    out_d = nc.dram_tensor("out", [C, 2048], BF16, kind="ExternalOutput")

    stats_in_d = nc.dram_tensor("stats_in", [128, 4], F32)
    use_ag = not os.environ.get("KERNEL_NOAG")
    if use_ag:
        stats_out_d = nc.dram_tensor("stats_out", [1024, 4], F32,
                                     addr_space="Shared")
    else:
        stats_out_d = nc.dram_tensor("stats_out", [128, 4], F32,
                                     addr_space="Shared")
    use_warm = not os.environ.get("KERNEL_NOWARM")
    if use_warm:
        warm_in_d = nc.dram_tensor("warm_in", [128, 1], F32)
        warm_out_d = nc.dram_tensor("warm_out", [128, 1], F32,
                                    addr_space="Shared")

    with tile.TileContext(nc) as tc:
        with (
            tc.tile_pool(name="consts", bufs=1) as consts,
            tc.tile_pool(name="work", bufs=1) as work,
            tc.tile_pool(name="persist", bufs=1) as persist,
        ):

            # long-lived activation tensors for the attention phase
            with tc.tile_pool(name="bigC", bufs=1) as bigC:
                xqr = [None, None]
                qrep = bigC.tile([128, WQ], F16, tag="qrep", name="qrep")
                krep = bigC.tile([128, N], F16, tag="krep", name="krep")
                vT = bigC.tile([128, 32, C + 2], BF16, tag="vT", name="vT")
                A = persist.tile([128, 2, C], F16, tag="A", name="A")
                grca = [None, None]

                # ---------- phase A/B: qkv gen, xT stream, channel attn ----------
                with (
                    tc.tile_pool(name="bigA", bufs=1) as bigA,
                    tc.tile_pool(name="psAB", bufs=1, space="PSUM") as psAB,
                ):
                    # big activations: xq chunked across rings on the sync
                    # queue, xb on the gpsimd (SWDGE) queue so descriptor
                    # generation runs in parallel
                    for ct in range(2):
                        xqr[ct] = bigC.tile([128, WQ], F16, tag=f"xqr{ct}",
                                            name=f"xqr{ct}")
                        for hf in range(2):
                            nc.sync.dma_start(
                                out=xqr[ct][:, 1088 * hf : 1088 * hf + 1088],
                                in_=xq_d[128 * ct : 128 * ct + 128,
                                         1088 * hf : 1088 * hf + 1088])
                    xr = [None, None]
                    for ct in range(2):
                        xr[ct] = bigA.tile([128, N], BF16, tag=f"xr{ct}",
                                           name=f"xr{ct}")
                    for ct in range(2):
                        for hf in range(2):
                            nc.gpsimd.dma_start(
                                out=xr[ct][:, 2048 * hf : 2048 * hf + 2048],
                                in_=xb_d[128 * ct : 128 * ct + 128,
                                         2048 * hf : 2048 * hf + 2048])

                    # packed consts on the scalar DMA queue (parallel head)
                    wq_sb = consts.tile([128, 256], F16, tag="wq", name="wq")
                    nc.scalar.dma_start(out=wq_sb[:], in_=wf16_d[:])
                    wbf_sb = consts.tile([128, 772], BF16, tag="wbf", name="wbf")
                    nc.scalar.dma_start(out=wbf_sb[:], in_=wbf_d[:])
                    spack = consts.tile([128, SP_COLS], F32, tag="spack",
                                        name="spack")
                    nc.scalar.dma_start(out=spack[:], in_=spack_d[:])
                    wqrep_r = [wq_sb[:, 128 * kt : 128 * kt + 128] for kt in range(2)]
                    wkrep_r = [wbf_sb[:, 128 * kt : 128 * kt + 128] for kt in range(2)]
                    wvt_r = [wbf_sb[:, 256 + (C + 2) * kt : 256 + (C + 2) * (kt + 1)]
                             for kt in range(2)]
                    bvtile = spack[:, SP_BV : SP_BV + C + 2]

                    # identities: bf16 pair for the xT matmuls, f32r for PE
                    # transposes
                    idf = work.tile([128, C], F32, tag="idf", name="idf")
                    nc.gpsimd.memset(idf[:], 0.0)
                    make_identity(nc, idf[:, 0:128], nomemset=True)
                    identr = [None, None]
                    identr[0] = consts.tile([128, C], BF16, tag="id0", name="id0")
                    nc.gpsimd.memset(identr[0][:, 128:256], 0.0)
                    nc.vector.tensor_copy(identr[0][:, 0:128], idf[:, 0:128])
                    identr[1] = consts.tile([128, C], BF16, tag="id1", name="id1")
                    nc.gpsimd.memset(identr[1][:, 0:128], 0.0)
                    nc.vector.tensor_copy(identr[1][:, 128:256], idf[:, 0:128])
                    idtr = consts.tile([128, 128], F32R, tag="idtr", name="idtr")
                    nc.vector.tensor_copy(idtr[:], idf[:, 0:128])

                    eps_sb = consts.tile([128, 1], F32, tag="eps", name="eps")
                    nc.vector.memset(eps_sb[:], BN_EPS)

                    # warm-up collective: pre-arms the CC subsystem so the
                    # real stats exchange at the end doesn't pay first-use
                    # latency; sequenced after the input-DMA burst
                    if use_warm:
                        warm = consts.tile([128, 1], F32, tag="warm", name="warm")
                        nc.vector.memset(warm[:], 0.0)
                        nc.gpsimd.dma_start(out=warm_in_d[:], in_=warm[:])
                        nc.gpsimd.collective_compute(
                            "AllReduce", ALU.add,
                            replica_groups=[list(range(8))],
                            ins=[warm_in_d[:]],
                            outs=[warm_out_d[:]],
                        )

                    # q/k generation (4x-replicated along d)
                    for off, cw in CHUNKS:
                        ps = psAB.tile([128, cw], F32, tag="qk", name="qk", bufs=2)
                        nc.tensor.matmul(ps[:], wqrep_r[0], xqr[0][:, off : off + cw],
                                         start=True, stop=False)
                        nc.tensor.matmul(ps[:], wqrep_r[1], xqr[1][:, off : off + cw],
                                         start=False, stop=True)
                        nc.scalar.activation(qrep[:, off : off + cw], ps[:], AF.Identity,
                                             bias=spack[:, SP_BQ : SP_BQ + 1],
                                             scale=0.25)
                    for kc in range(8):
                        off = 512 * kc
                        ps = psAB.tile([128, 512], F32, tag="qk", name="qk", bufs=2)
                        nc.tensor.matmul(ps[:], wkrep_r[0], xr[0][:, off : off + 512],
                                         start=True, stop=False)
                        nc.tensor.matmul(ps[:], wkrep_r[1], xr[1][:, off : off + 512],
                                         start=False, stop=True)
                        nc.scalar.activation(krep[:, off : off + 512], ps[:], AF.Identity,
                                             bias=spack[:, SP_BK : SP_BK + 1],
                                             scale=1.0)

                    # vT gen + streamed xT -> channel-attention energy
                    ec = [psAB.tile([128, C], F32, tag=f"ec{ih}", name=f"ec{ih}")
                          for ih in range(2)]
                    for nt in range(32):
                        sl = slice(128 * nt, 128 * nt + 128)
                        ps = psAB.tile([128, C + 2], F32, tag="vx", name="vx", bufs=3)
                        nc.tensor.matmul(ps[:], xr[0][:, sl], wvt_r[0],
                                         start=True, stop=False)
                        nc.tensor.matmul(ps[:], xr[1][:, sl], wvt_r[1],
                                         start=False, stop=True)
                        nc.vector.tensor_add(vT[:, nt, :], ps[:], bvtile)
                        ps2 = psAB.tile([128, C], F32, tag="vx", name="vx", bufs=3)
                        nc.tensor.matmul(ps2[:], xr[0][:, sl], identr[0][:],
                                         start=True, stop=False)
                        nc.tensor.matmul(ps2[:], xr[1][:, sl], identr[1][:],
                                         start=False, stop=True)
                        xTn = bigA.tile([128, C], BF16, tag="xTn", name="xTn", bufs=2)
                        nc.vector.tensor_copy(xTn[:], ps2[:])
                        for ih in range(2):
                            nc.tensor.matmul(ec[ih][:],
                                             xTn[:, 128 * ih : 128 * ih + 128], xTn[:],
                                             start=(nt == 0), stop=(nt == 31))

                    # channel-attention softmax (row-wise) + transpose
                    U = [None, None]
                    for ih in range(2):
                        negmax = work.tile([128, 1], F32, tag="negmax", name="negmax")
                        nc.vector.reduce_max(negmax[:], ec[ih][:], axis=AX.X, negate=True)
                        U[ih] = bigA.tile([128, C], F32R, tag=f"U{ih}", name=f"U{ih}")
                        nc.scalar.activation(U[ih][:], ec[ih][:], AF.Exp,
                                             bias=negmax[:], scale=1.0)
                        ssum = work.tile([128, 1], F32, tag="ssum", name="ssum")
                        nc.vector.reduce_sum(ssum[:], U[ih][:], axis=AX.X)
                        rc = work.tile([128, 1], F32, tag="rc", name="rc")
                        nc.vector.reciprocal(rc[:], ssum[:])
                        grca[ih] = consts.tile([128, 1], F32, tag=f"grca{ih}", name=f"grca{ih}")
                        nc.vector.tensor_scalar_mul(grca[ih][:], rc[:], gca)
                    for ih in range(2):
                        for jt in range(2):
                            trp = psAB.tile([128, 128], F32R, tag="tr", name="tr")
                            nc.tensor.transpose(trp[:], U[ih][:, 128 * jt : 128 * jt + 128],
                                                idtr[:])
                            nc.vector.tensor_copy(A[:, jt, 128 * ih : 128 * ih + 128],
                                                  trp[:])

                # conv weights: bf16 straight from HBM; prefetches during C
                wf_sb = [None] * 4
                for it in range(4):
                    wf_sb[it] = persist.tile([128, 2304], BF16, tag=f"wf{it}",
                                             name=f"wf{it}")
                    nc.sync.dma_start(out=wf_sb[it][:], in_=wft_d[it])

                # ---------- phase C: channel-attn apply + position attention ----------
                pad = [persist.tile([128, WROWS, W + 2], BF16, tag=f"pad{t}", name=f"pad{t}")
                       for t in range(4)]
                # only the two border columns need zeroing: the finalize ops
                # write every interior column of all 34 rows
                zcol = work.tile([128, WROWS, 1], F32, tag="zcol", name="zcol")
                nc.vector.memset(zcol[:], 0.0)
                for t in range(4):
                    nc.vector.tensor_copy(pad[t][:, :, 0:1], zcol[:])
                    nc.vector.tensor_copy(pad[t][:, :, W + 1 : W + 2], zcol[:])

                with (
                    tc.tile_pool(name="psC", bufs=1, space="PSUM") as psC,
                    tc.tile_pool(name="ptp", bufs=2) as ptp,
                ):
                    # ca = (U @ xq) * (gamma_ca / rowsum) + xq, into padded tiles
                    for ih in range(2 * (not os.environ.get("KERNEL_SKIP_CA"))):
                        isl = slice(128 * ih, 128 * ih + 128)
                        for off, cw in CHUNKS:
                            rows = cw // W
                            roff = off // W
                            ca = psC.tile([128, cw], F32, tag=f"pa{ih}", name=f"pa{ih}")
                            nc.tensor.matmul(ca[:], A[:, 0, isl], xqr[0][:, off : off + cw],
                                             start=True, stop=False)
                            nc.tensor.matmul(ca[:], A[:, 1, isl], xqr[1][:, off : off + cw],
                                             start=False, stop=True)
                            nc.vector.scalar_tensor_tensor(
                                out=pad[2 + ih][:, roff : roff + rows, 1 : 1 + W],
                                in0=ca[:].rearrange("p (r w) -> p r w", w=W),
                                scalar=grca[ih][:],
                                in1=xqr[ih][:, off : off + cw]
                                    .rearrange("p (r w) -> p r w", w=W),
                                op0=ALU.mult, op1=ALU.add,
                            )

                    for off, cw in ([] if os.environ.get("KERNEL_SKIP_C") else CHUNKS):
                        rows = cw // W
                        roff = off // W
                        PT = ptp.tile([128, 32, 512], BF16, tag="pt", name="pt")
                        for jt in range(32):
                            en = psC.tile([128, cw], F32, tag=f"en{jt % 4}",
                                          name=f"en{jt % 4}")
                            nc.tensor.matmul(
                                en[:],
                                krep[:, 128 * jt : 128 * jt + 128],
                                qrep[:, off : off + cw],
                                start=True, stop=True,
                            )
                            nc.scalar.activation(PT[:, jt, 0:cw], en[:], AF.Exp,
                                                 bias=0.0, scale=1.0)

                        for ib in range(cw // 128):
                            gib = off // 128 + ib
                            paps = psC.tile([128, C + 2], F32, tag=f"pa{ib % 3}",
                                            name=f"pa{ib % 3}")
                            for jt in range(32):
                                nc.tensor.matmul(
                                    paps[:],
                                    PT[:, jt, 128 * ib : 128 * ib + 128],
                                    vT[:, jt, :],
                                    start=(jt == 0), stop=(jt == 31),
                                )
                            recip = work.tile([128, 1], F32, tag="recip",
                                              name="recip", bufs=2)
                            nc.vector.reciprocal(recip[:], paps[:, C : C + 1])
                            grm = work.tile([128, 1], F32, tag="grm", name="grm",
                                            bufs=2)
                            nc.vector.tensor_scalar(
                                out=grm[:], in0=recip[:],
                                scalar1=spack[:, SP_MASK + gib : SP_MASK + gib + 1],
                                scalar2=gpa,
                                op0=ALU.mult, op1=ALU.mult)
                            tsc = work.tile([128, C], BF16, tag="tsc", name="tsc",
                                            bufs=2)
                            nc.vector.tensor_scalar_mul(tsc[:], paps[:, 0:C], grm[:])
                            for ch in range(2):
                                trp = psC.tile([128, 128], BF16, tag="tr", name="tr")
                                nc.tensor.transpose(
                                    trp[:], tsc[:, 128 * ch : 128 * ch + 128],
                                    identr[0][:, 0:128])
                                r2 = 128 // W
                                r0 = gib * r2
                                nc.vector.tensor_add(
                                    pad[ch][:, r0 : r0 + r2, 1 : 1 + W],
                                    trp[:].rearrange("p (r w) -> p r w", w=W),
                                    xqr[ch][:, 128 * gib : 128 * gib + 128]
                                        .rearrange("p (r w) -> p r w", w=W),
                                )

            # ---------- phase D: 3x3 conv + BN stats ----------
            y_sb = [persist.tile([128, 2048], F32, tag=f"ysb{o}", name=f"ysb{o}")
                    for o in range(2)]
            if os.environ.get("KERNEL_SKIP_D"):
                for o in range(2):
                    nc.vector.memset(y_sb[o][:], 0.0)
            sums = [consts.tile([128, 4], F32, tag=f"sums{o}", name=f"sums{o}")
                    for o in range(2)]
            sqs = [consts.tile([128, 4], F32, tag=f"sqs{o}", name=f"sqs{o}")
                   for o in range(2)]
            if os.environ.get("KERNEL_SKIP_D"):
                for o in range(2):
                    nc.vector.memset(sums[o][:], 0.0)
                    nc.vector.memset(sqs[o][:], 0.0)

            # stats layout: cols [sum_oh0, sum_oh1, sq_oh0, sq_oh1]
            stats_sb = consts.tile([128, 4], F32, tag="stats", name="stats")
            if os.environ.get("KERNEL_SKIP_D"):
                nc.vector.memset(stats_sb[:], 0.0)

            with (
                tc.tile_pool(name="psD", bufs=4, space="PSUM") as psD,
            ):
                for oh in range(2 * (not os.environ.get("KERNEL_SKIP_D"))):
                    for pc in range(4):
                        yps = psD.tile([128, 512], F32, tag="y", name="y")
                        first = True
                        for it in range(4):
                            for dy in range(3):
                                for dx in range(3):
                                    wslice = slice(
                                        ((dy * 3 + dx) * 2 + oh) * 128,
                                        ((dy * 3 + dx) * 2 + oh) * 128 + 128,
                                    )
                                    last = (it == 3 and dy == 2 and dx == 2)
                                    rhs = pad[it][:, 8 * pc + dy : 8 * pc + dy + 8,
                                                  dx : dx + W]
                                    nc.tensor.matmul(
                                        yps[:], wf_sb[it][:, wslice], rhs,
                                        start=first, stop=last,
                                    )
                                    first = False
                        # stats ops first: they gate the collective, the
                        # y_sb copy only gates the (post-collective) output
                        nc.vector.reduce_sum(sums[oh][:, pc : pc + 1], yps[:],
                                             axis=AX.X)
                        dscr = work.tile([128, 512], F32, tag="dscr", name="dscr", bufs=2)
                        nc.scalar.activation(dscr[:], yps[:], AF.Square,
                                             accum_out=sqs[oh][:, pc : pc + 1])
                        ysl = y_sb[oh][:, 512 * pc : 512 * pc + 512]
                        nc.vector.tensor_copy(ysl, yps[:])

                    nc.vector.reduce_sum(stats_sb[:, oh : oh + 1],
                                         sums[oh][:], axis=AX.X)
                    nc.vector.reduce_sum(stats_sb[:, 2 + oh : 3 + oh],
                                         sqs[oh][:], axis=AX.X)

            nc.sync.dma_start(out=stats_in_d[:], in_=stats_sb[:])
            allst = consts.tile([128, 4], F32, tag="allst", name="allst")
            if use_ag:
                nc.gpsimd.collective_compute(
                    "AllGather", ALU.bypass,
                    replica_groups=[list(range(8))],
                    ins=[stats_in_d[:]],
                    outs=[stats_out_d[:]],
                )
                gsb = consts.tile([128, 8, 4], F32, tag="gsb", name="gsb")
                nc.sync.dma_start(
                    out=gsb[:],
                    in_=stats_out_d[:].rearrange("(r p) c -> p r c", p=128))
                gview = gsb[:].rearrange("p r c -> p c r")
                nc.vector.reduce_sum(allst[:], gview, axis=AX.X)
            else:
                nc.gpsimd.collective_compute(
                    "AllReduce", ALU.add,
                    replica_groups=[list(range(8))],
                    ins=[stats_in_d[:]],
                    outs=[stats_out_d[:]],
                )
                nc.sync.dma_start(out=allst[:], in_=stats_out_d[:])

            # per-oh scale/shift, both halves at once
            meanex = work.tile([128, 4], F32, tag="meanex", name="meanex")
            nc.vector.tensor_scalar_mul(meanex[:], allst[:], 1.0 / NPOS)
            mean2 = meanex[:, 0:2]
            ex2 = meanex[:, 2:4]
            msq2 = work.tile([128, 2], F32, tag="msq2", name="msq2")
            nc.vector.tensor_mul(msq2[:], mean2, mean2)
            var2 = work.tile([128, 2], F32, tag="var2", name="var2")
            nc.vector.tensor_sub(var2[:], ex2, msq2[:])
            std2 = work.tile([128, 2], F32, tag="std2", name="std2")
            nc.scalar.activation(std2[:], var2[:], AF.Sqrt, bias=eps_sb[:], scale=1.0)
            rstd2 = work.tile([128, 2], F32, tag="rstd2", name="rstd2")
            nc.vector.reciprocal(rstd2[:], std2[:])
            scale2 = work.tile([128, 2], F32, tag="scale2", name="scale2")
            nc.vector.tensor_mul(scale2[:], spack[:, SP_BNG : SP_BNG + 2], rstd2[:])
            tmp2 = work.tile([128, 2], F32, tag="tmp2", name="tmp2")
            nc.vector.tensor_mul(tmp2[:], mean2, scale2[:])
            shift2 = work.tile([128, 2], F32, tag="shift2", name="shift2")
            nc.vector.tensor_sub(shift2[:], spack[:, SP_BNB : SP_BNB + 2], tmp2[:])

            # normalize split across engines: scalar handles oh=0, vector
            # handles oh=1 (mult-add then max-with-0), DMAs on both queues
            for pc in range(2):
                osb = work.tile([128, 1024], BF16, tag="osb", name="osb", bufs=2)
                nc.scalar.activation(osb[:],
                                     y_sb[0][:, 1024 * pc : 1024 * pc + 1024],
                                     AF.Relu, bias=shift2[:, 0:1],
                                     scale=scale2[:, 0:1])
                nc.sync.dma_start(
                    out=out_d[0:128, 1024 * pc : 1024 * pc + 1024],
                    in_=osb[:],
                )
                vsc = work.tile([128, 1024], F32, tag="vsc", name="vsc", bufs=2)
                nc.vector.tensor_scalar(
                    out=vsc[:], in0=y_sb[1][:, 1024 * pc : 1024 * pc + 1024],
                    scalar1=scale2[:, 1:2], scalar2=shift2[:, 1:2],
                    op0=ALU.mult, op1=ALU.add)
                osbv = work.tile([128, 1024], BF16, tag="osbv", name="osbv", bufs=2)
                nc.vector.tensor_scalar_max(osbv[:], vsc[:], 0.0)
                nc.scalar.dma_start(
                    out=out_d[128:256, 1024 * pc : 1024 * pc + 1024],
                    in_=osbv[:],
                )

    nc.compile()
    return nc


def _ensure_trace_hook():
    try:
        import antenv.axon_hooks  # noqa: F401
        return
    except ImportError:
        pass
    try:
        from trn_agent_boot.trn_boot import _ntff_profile_via_ctypes
    except ImportError:
        return
    mod = types.ModuleType("antenv.axon_hooks")
    try:
        hook = _ntff_profile_via_ctypes("/opt/axon/libaxon_pjrt.so")
    except Exception:
        return
    mod.get_axon_ntff_profile_hook = lambda: hook
    mod.set_axon_ntff_profile_hook = lambda h: None
    sys.modules["antenv.axon_hooks"] = mod


def kernel(x, wq, bq, wk, bk, wv, bv, gamma_pa, gamma_ca, wf, bn_gamma, bn_beta):
    x = np.ascontiguousarray(np.asarray(x, np.float32))
    wq = np.asarray(wq, np.float32)
    bq = np.asarray(bq, np.float32)
    wk = np.asarray(wk, np.float32)
    bk = np.asarray(bk, np.float32)
    wv = np.asarray(wv, np.float32)
    bv = np.asarray(bv, np.float32)
    wf = np.asarray(wf, np.float32)
    gpa = float(np.asarray(gamma_pa).reshape(-1)[0])
    gca = float(np.asarray(gamma_ca).reshape(-1)[0])
    bn_gamma = np.asarray(bn_gamma, np.float32)
    bn_beta = np.asarray(bn_beta, np.float32)

    nc = _build(gpa, gca)

    # shared (per-core-identical) prepped weights
    # wf16: 4x-replicated wq^T tiles side by side
    wqrep = np.tile(wq.T, (1, 4))                                # [256, 128]
    wf16 = np.ascontiguousarray(
        np.concatenate([wqrep[0:128], wqrep[128:256]], axis=1)
    ).astype(np.float16)                                         # [128, 256]
    # wbf: wk tiles then augmented-wv tiles
    wkrep = np.tile(wk.T, (1, 4))
    wvt = np.concatenate([wv.T, np.zeros((C, 2), np.float32)], axis=1)
    wbf = np.ascontiguousarray(
        np.concatenate([wkrep[0:128], wkrep[128:256],
                        wvt[0:128], wvt[128:256]], axis=1)
    ).astype(ml_dtypes.bfloat16)                                 # [128, 772]
    # wft[it, i, (dy dx o)] = wf[o, 128*it + i, dy, dx]
    wft = np.ascontiguousarray(
        wf.reshape(C, 4, 128, 3, 3).transpose(1, 2, 3, 4, 0).reshape(4, 128, 2304)
    ).astype(ml_dtypes.bfloat16)

    # f32 scalar pack; mask column differs per core (filled below)
    spack = np.zeros((128, SP_COLS), np.float32)
    # q is generated pre-scaled by 1/4: the energy matmul contracts over a
    # 4x-replicated K=128, summing each q.k product four times
    spack[:, SP_BQ] = np.tile(bq, 4) / 4.0
    spack[:, SP_BK] = np.tile(bk, 4)
    spack[:, SP_BNG] = bn_gamma[0:128]
    spack[:, SP_BNG + 1] = bn_gamma[128:256]
    spack[:, SP_BNB] = bn_beta[0:128]
    spack[:, SP_BNB + 1] = bn_beta[128:256]
    # augmented V bias row: column 256 is all-ones, so the PV matmul's
    # column 256 yields the softmax denominator
    bvrow = np.concatenate([bv, np.ones(1, np.float32), np.zeros(1, np.float32)])
    spack[:, SP_BV : SP_BV + C + 2] = bvrow[None, :]

    in_maps = []
    for core in range(8):
        b, hf = divmod(core, 2)
        r0 = hf * 32
        e0 = r0 - 1
        xqw = np.zeros((C, WROWS, W), np.float32)
        msk = np.zeros((WROWS, W), np.float32)
        lo, hi = max(e0, 0), min(e0 + WROWS, H)
        xqw[:, lo - e0 : hi - e0, :] = x[b][:, lo:hi, :]
        msk[lo - e0 : hi - e0, :] = 1.0
        spc = spack.copy()
        spc[:, SP_MASK : SP_MASK + 17] = msk.reshape(17, 128).T
        in_maps.append({
            "xq": np.ascontiguousarray(xqw.reshape(C, WQ)).astype(np.float16),
            "xb": np.ascontiguousarray(x[b].reshape(C, N)).astype(ml_dtypes.bfloat16),
            "wf16": wf16,
            "wbf": wbf,
            "spack": np.ascontiguousarray(spc),
            "wft": wft,
        })

    trace = bool(os.environ.get("BASS_TRACE"))
    if trace:
        _ensure_trace_hook()
    res = run_bass_kernel_spmd(nc, in_maps, list(range(8)), trace=trace)
    LAST_RESULT["exec_time_ns"] = res.exec_time_ns
    LAST_RESULT["mean_exec_time_ns"] = res.mean_exec_time_ns

    out = np.empty((B, C, H, W), np.float32)
    for core in range(8):
        b, hf = divmod(core, 2)
        out[b][:, 32 * hf : 32 * hf + 32, :] = (
            res.results[core]["out"].astype(np.float32).reshape(C, 32, W)
        )
    return out
